# revision 9
# baseline (speedup 1.0000x reference)
"""Multi-head attention (b=2, n=2048, dim=1024, h=16, fp32) on 8 TRN2 NeuronCores.

Sharding: 2 batches x 4 head-groups (4 heads each). Each core computes, for its
batch element and 4 heads: QKV projection, softmax attention, and a partial
output projection (W_out rows of its heads). Host sums the 4 partials per batch
and adds the bias.

Device layout choices (per core):
  - x arrives pre-transposed (host) as xT [128, 8, 2048]  (p=dim%128, kc=dim//128, n)
  - Q^T/K^T computed as [128, 2, 2048]: pair g holds heads 2g (partitions 0-63)
    and 2g+1 (partitions 64-127); row r of pair g = W_qkv column (g*128+r).
  - S^T = K @ Q^T per head via row-tiled (K=64) matmul pairs (the pair runs
    concurrently on the two array row-halves: ~216ns/pair); softmax exp on
    ScalarE directly PSUM->SBUF with scale=dim^-0.5 folded in (no max
    subtraction needed: |scores*scale| < ~0.5).
  - V is augmented with a ones column per head ([V_h | 1]) so the PV matmul's
    65th output row accumulates the softmax denominator for free.
  - Normalization: reciprocal_approx_fast (DVE) + partition_broadcast (GPSIMD)
    + one tensor_tensor multiply fused with the PSUM->SBUF evacuation.
  - Emission schedule: the QKV/out projections are diced into granules of <=4
    matmuls and drizzled into the attention sweeps (one granule per chunk) so
    the PE never starves ScalarE's exp stream and projection LDWEIGHTS mostly
    hide behind adjacent matmuls. Attention starts as soon as the first
    512-token block of x lands; out-projection for block b fills the sweeps
    after its pair-1 sweep completes, leaving only the last block as tail.
  - Optionally every EXP_DVE_EVERY-th chunk computes exp on the Vector engine
    via ((x+2)^2/8 + 0.5)^2 (max rel err ~0.6% on |x|<=0.5) to unload ScalarE.
"""

import os
import numpy as np
from collections import deque
from contextlib import ExitStack
from functools import partial

import concourse.bass as bass
import concourse.mybir as mybir
import concourse.tile as tile
from concourse import bacc
from concourse.bass import ts
from concourse.bass_utils import run_bass_kernel_spmd

F32 = mybir.dt.float32
F32R = mybir.dt.float32r

N_CORES = 8
HEADS = 16
DH = 64  # head dim


class Cfg:
    def __init__(self, n, dim, hg):
        self.n = n                    # sequence length (per core)
        self.dim = dim                # model dim
        self.hg = hg                  # heads per core
        self.kc = dim // 128          # dim chunks of 128
        self.nqb = max(1, n // 512)   # query blocks of 512
        self.qb = min(n, 512)
        self.nkc = n // 128           # key chunks of 128
        self.pairs = hg // 2
        self.shard = hg * DH          # qkv shard columns per section
        self.vw = hg * (DH + 1)       # V columns incl per-head ones col
        self.mm_dt = mybir.dt.float16
        self.np_dt = np.float16


FULL = Cfg(2048, 1024, 4)
# in pair-1 sweeps, every n-th chunk's exp runs on DVE instead of ScalarE (0 = off)
EXP_DVE_EVERY = int(os.environ.get("ATTN_EXP_DVE_EVERY", "0"))


def build_kernel(tc, ctx, cfg, xT, wq, wk, wv, wo, out):
    nc = tc.nc
    P = 128
    KC, NQB, QB, NKC, PAIRS = cfg.kc, cfg.nqb, cfg.qb, cfg.nkc, cfg.pairs
    MD = cfg.mm_dt
    SCALE = cfg.dim ** -0.5
    M_SLABS = cfg.shard // 128  # = PAIRS

    wpool = ctx.enter_context(tc.tile_pool(name="w", bufs=1))
    wq_sb = wpool.tile([P, KC, cfg.shard], MD, tag="wq", name="wq_sb")
    wk_sb = wpool.tile([P, KC, cfg.shard], MD, tag="wk", name="wk_sb")
    wv_sb = wpool.tile([P, KC, cfg.vw], MD, tag="wv", name="wv_sb")
    wo_sb = wpool.tile([P, M_SLABS, cfg.dim], MD, tag="wo", name="wo_sb")

    per = ctx.enter_context(tc.tile_pool(name="per", bufs=1))
    qt = {}  # (pair, nqb) -> [128, QB]
    kt = {}
    vt = {}  # nt -> [128, vw]
    on = {}  # (slab, nqb) -> [128, QB]  normalized O^T for out-proj lhsT
    for g in range(PAIRS):
        for b in range(NQB):
            qt[g, b] = per.tile([P, QB], MD, tag=f"qt{g}_{b}", name=f"qt{g}_{b}")
            kt[g, b] = per.tile([P, QB], MD, tag=f"kt{g}_{b}", name=f"kt{g}_{b}")
            on[g, b] = per.tile([P, QB], MD, tag=f"on{g}_{b}", name=f"on{g}_{b}")
    for t in range(NKC):
        vt[t] = per.tile([P, cfg.vw], MD, tag=f"v{t}", name=f"v{t}")

    xpool = ctx.enter_context(tc.tile_pool(name="x", bufs=1))
    # single-buffer projection psum pools: an emit's accumulation spans several
    # interleaved granules, so each class gets a dedicated bank (no rotation
    # collisions). psS 4 + psO 2 + paA 1 + paV 1 = 8 banks.
    paA = ctx.enter_context(tc.tile_pool(name="paA", bufs=1, space="PSUM"))
    paV = ctx.enter_context(tc.tile_pool(name="paV", bufs=1, space="PSUM"))
    psS = ctx.enter_context(tc.tile_pool(name="psS", bufs=2, space="PSUM"))
    psO = ctx.enter_context(tc.tile_pool(name="psO", bufs=1, space="PSUM"))
    epool = ctx.enter_context(tc.tile_pool(name="e", bufs=8))
    ppool = ctx.enter_context(tc.tile_pool(name="p", bufs=2))
    npool = ctx.enter_context(tc.tile_pool(name="nrm", bufs=3))
    copool = ctx.enter_context(tc.tile_pool(name="co", bufs=6))

    xts = {}
    for b in range(NQB):
        xts[b] = xpool.tile([P, KC, QB], MD, tag=f"xt{b}", name=f"xt{b}")

    # ---- input DMA: first x block + wk lead; everything else follows in
    # halves so the 16 queues run wide and block 0 lands first.
    h = KC // 2
    for q4 in range(4):
        nc.sync.dma_start(wk_sb[:, ts(q4, 2)], wk[:, ts(q4, 2)])
    for q8 in range(8):
        nc.sync.dma_start(xts[0][:, q8], xT[:, q8, ts(0, QB)])
    for q4 in range(4):
        nc.sync.dma_start(wq_sb[:, ts(q4, 2)], wq[:, ts(q4, 2)])
    for q4 in range(4):
        nc.sync.dma_start(wv_sb[:, ts(q4, 2)], wv[:, ts(q4, 2)])
    for b in range(1, NQB):
        nc.sync.dma_start(xts[b][:, :h], xT[:, :h, ts(b, QB)])
        nc.sync.dma_start(xts[b][:, h:], xT[:, h:, ts(b, QB)])
    nc.sync.dma_start(wo_sb[:], wo[:])

    # ---- granule machinery: pend holds (key, fn) units of <=4 PE matmuls.
    pend = deque()
    emitted = set()

    def fill(budget):
        n = 0
        while pend and n < budget:
            key, fn = pend.popleft()
            fn()
            emitted.add(key)
            n += 1

    def require(key):
        while pend and key not in emitted:
            k, fn = pend.popleft()
            fn()
            emitted.add(k)

    # ---- projection emitters (as granules) ----
    qk_ps = {}

    def qk_part(w_sb, dst, g, b, kcs, evac):
        if kcs[0] == 0:
            qk_ps[0] = paA.tile([P, 512], F32, tag="pa", name="pa")
        ps = qk_ps[0]
        for kc in kcs:
            nc.tensor.matmul(
                ps[:, :QB],
                lhsT=w_sb[:, kc, ts(g, 128)],
                rhs=xts[b][:, kc, :],
                start=(kc == 0),
                stop=(kc == KC - 1),
            )
        if evac:
            nc.vector.tensor_copy(dst[g, b][:], ps[:, :QB])

    def push_qk(which, w_sb, dst, g, b):
        groups = [list(range(0, 3)), list(range(3, 6)), list(range(6, KC))]
        for i, kcs in enumerate(groups):
            key = (which, g, b) if i == len(groups) - 1 else (which, g, b, i)
            pend.append((key, partial(qk_part, w_sb, dst, g, b, kcs, i == len(groups) - 1)))

    v_ps = {}

    def v_part(nt, kcs, evac):
        vb, t = divmod(nt, QB // 128)
        if kcs[0] == 0:
            v_ps[0] = paV.tile([P, 512], F32, tag="pv", name="pv")
        ps = v_ps[0]
        for kc in kcs:
            nc.tensor.matmul(
                ps[:, : cfg.vw],
                lhsT=xts[vb][:, kc, ts(t, 128)],
                rhs=wv_sb[:, kc, :],
                start=(kc == 0),
                stop=(kc == KC - 1),
            )
        if evac:
            nc.vector.tensor_copy(vt[nt][:], ps[:, : cfg.vw])
            v4 = vt[nt][:].rearrange("p (h e) -> p h e", e=DH + 1)
            nc.vector.memset(v4[:, :, DH : DH + 1], 1.0)

    def emit_v(nt):
        v_part(nt, list(range(0, 4)), False)
        v_part(nt, list(range(4, KC)), True)

    # ---- out projection granules: per (bb, t, nh): 2 matmuls + evac + DMA
    def out_part(bb, t, nh, tail):
        nt = bb * (QB // 128) + t
        # alternate between the two projection psum banks (V emits are long
        # done by now) so consecutive out granules pipeline instead of
        # serializing on a single bank's evacuation
        if (t * (cfg.dim // 512) + nh) % 2 == 0:
            ps = paA.tile([P, 512], F32, tag="pa", name="pc")
        else:
            ps = paV.tile([P, 512], F32, tag="pv", name="pcv")
        for kc in range(M_SLABS):
            nc.tensor.matmul(
                ps[:],
                lhsT=on[kc, bb][:, ts(t, 128)],
                rhs=wo_sb[:, kc, ts(nh, 512)],
                start=(kc == 0),
                stop=(kc == M_SLABS - 1),
            )
        ot = copool.tile([P, 512], MD, tag="ot", name="ot")
        if tail:
            # tail blocks: ScalarE is idle after the last exp
            nc.scalar.copy(ot[:], ps[:])
        else:
            nc.vector.tensor_copy(ot[:], ps[:])
        for p4 in range(4):
            nc.sync.dma_start(
                out[ts(nt, 128)][ts(p4, 32), ts(nh, 512)], ot[ts(p4, 32), :]
            )

    def push_out(bb, tail=False):
        for t in range(QB // 128):
            for nh in range(cfg.dim // 512):
                pend.append(((("o", bb, t, nh)), partial(out_part, bb, t, nh, tail)))

    # ---- attention sweep ----
    sweep_idx = [0]

    def attention(b, g, with_v=False, fe=2, dve_every=0, last=False):
        o_ps = psO.tile([P, 2, 512], F32, tag="o", name="o_ps")
        require(("q", g, b))
        ets = {}

        def pv(c):
            v4 = vt[c][:].rearrange("p (h e) -> p h e", e=DH + 1)
            e_t = ets.pop(c)
            for a in range(2):
                hh = 2 * g + a
                nc.tensor.matmul(
                    o_ps[0 : DH + 1, a, :QB],
                    lhsT=v4[:, hh, :],
                    rhs=e_t[:, a, :QB],
                    start=(c == 0),
                    stop=(c == NKC - 1),
                )

        for c in range(NKC):
            require(("k", g, c * 128 // QB))
            s_ps = psS.tile([P, 2, 512], F32, tag="s", name="s_ps")
            for a in range(2):
                lo = a * 64
                nc.tensor.matmul(
                    s_ps[:, a, :QB],
                    lhsT=kt[g, c * 128 // QB][lo : lo + 64, ts(c % (QB // 128), 128)],
                    rhs=qt[g, b][lo : lo + 64, :],
                    start=True,
                    stop=True,
                )
            # PV for the previous chunk goes to the PE queue here, ahead of
            # this chunk's exp, hiding the exp->PV semaphore handoff
            if c > 0:
                pv(c - 1)
            e_t = epool.tile([P, 2, 512], MD, tag="e", name="e_t")
            ets[c] = e_t
            if dve_every and c % dve_every == dve_every - 1:
                # e = ((x+2)^2 / 8 + 0.5)^2, x = scale * s
                a_t = ppool.tile([P, 2, 512], MD, tag="pe_a", name="pe_a")
                b_t = ppool.tile([P, 2, 512], MD, tag="pe_b", name="pe_b")
                nc.vector.tensor_scalar(
                    a_t[:, :, :QB], s_ps[:, :, :QB], SCALE, 2.0,
                    mybir.AluOpType.mult, mybir.AluOpType.add,
                )
                nc.vector.tensor_tensor(
                    b_t[:, :, :QB], a_t[:, :, :QB], a_t[:, :, :QB], mybir.AluOpType.mult
                )
                nc.vector.tensor_scalar(
                    a_t[:, :, :QB], b_t[:, :, :QB], 0.125, 0.5,
                    mybir.AluOpType.mult, mybir.AluOpType.add,
                )
                nc.vector.tensor_tensor(
                    e_t[:, :, :QB], a_t[:, :, :QB], a_t[:, :, :QB], mybir.AluOpType.mult
                )
            else:
                nc.scalar.activation(
                    e_t[:, :, :QB],
                    s_ps[:, :, :QB],
                    mybir.ActivationFunctionType.Exp,
                    scale=SCALE,
                )
            if with_v and c < NKC - 1:
                emit_v(c + 1)
            if c % (2 * fe) == 2 * fe - 1:
                fill(2)
        pv(NKC - 1)
        sweep_idx[0] += 1
        # one-shot evacuation frees the O psum banks immediately; skipped on
        # the final sweep where nothing waits on the banks
        if last:
            oev = o_ps
        else:
            oev = npool.tile([P, 2, 512], F32, tag="oev", name="oev")
            nc.vector.tensor_copy(oev[0 : DH + 1, :, :QB], o_ps[0 : DH + 1, :, :QB])
        # normalize; stage the denom row at partition 0 (the custom DVE
        # reciprocal misreads inputs at a nonzero base partition). On the last
        # sweep run it in two q-halves so the tail out-proj (which consumes
        # 128-token slabs of `on`) can start after the first half.
        drow = npool.tile([1, 2, 512], F32, tag="drow", name="drow")
        recip = npool.tile([1, 2, 512], F32, tag="recip", name="recip")
        bcast = npool.tile([64, 2, 512], F32, tag="bcast", name="bcast")
        halves = [(0, QB // 2), (QB // 2, QB)] if last else [(0, QB)]
        for qlo, qhi in halves:
            nc.vector.tensor_copy(drow[:, :, qlo:qhi], o_ps[DH : DH + 1, :, qlo:qhi])
            nc.vector.reciprocal_approx_fast(
                out=recip[:, :, qlo:qhi], in_=drow[:, :, qlo:qhi]
            )
            nc.gpsimd.partition_broadcast(bcast[:, :, qlo:qhi], recip[:, :, qlo:qhi])
            for a in range(2):
                nc.vector.tensor_tensor(
                    on[g, b][a * 64 : a * 64 + 64, qlo:qhi],
                    oev[0:DH, a, qlo:qhi],
                    bcast[:, a, qlo:qhi],
                    mybir.AluOpType.mult,
                )

    # ---- emission schedule ----
    # seed: K(0,0) + Q(0,0) emit immediately (block 0 DMA leads); the rest of
    # the projections go through the granule queue, popped one per chunk and
    # force-drained by require() at dependency edges.
    push_qk("k", wk_sb, kt, 0, 0)
    push_qk("q", wq_sb, qt, 0, 0)
    for b in range(1, NQB):
        push_qk("k", wk_sb, kt, 0, b)
    for b in range(1, NQB):
        push_qk("q", wq_sb, qt, 0, b)
    for b in range(NQB):
        push_qk("k", wk_sb, kt, 1, b)
    for b in range(NQB):
        push_qk("q", wq_sb, qt, 1, b)

    emit_v(0)
    attention(0, 0, with_v=True, fe=4)
    for b in range(1, NQB):
        attention(b, 0, fe=3)
    for b in range(NQB):
        attention(b, 1, fe=2, dve_every=EXP_DVE_EVERY, last=(b == NQB - 1))
        # out-proj for block b becomes available now; queue it as filler work
        push_out(b, tail=(b == NQB - 1))
    while pend:
        fill(len(pend))


def build_program(cfg, num_devices=N_CORES):
    nc = bacc.Bacc("TRN2", target_bir_lowering=False, debug=False, num_devices=num_devices)
    P = 128
    xT = nc.dram_tensor("xT", [P, cfg.kc, cfg.n], cfg.mm_dt, kind="ExternalInput").ap()
    wq = nc.dram_tensor("wq", [P, cfg.kc, cfg.shard], cfg.mm_dt, kind="ExternalInput").ap()
    wk = nc.dram_tensor("wk", [P, cfg.kc, cfg.shard], cfg.mm_dt, kind="ExternalInput").ap()
    wv = nc.dram_tensor("wv", [P, cfg.kc, cfg.vw], cfg.mm_dt, kind="ExternalInput").ap()
    wo = nc.dram_tensor("wo", [P, cfg.shard // 128, cfg.dim], cfg.mm_dt, kind="ExternalInput").ap()
    out = nc.dram_tensor("out", [cfg.n, cfg.dim], cfg.mm_dt, kind="ExternalOutput").ap()
    with tile.TileContext(nc) as tc, ExitStack() as ctx:
        build_kernel(tc, ctx, cfg, xT, wq, wk, wv, wo, out)
    nc.compile()
    return nc


def shard_inputs(cfg, x, W_qkv, W_out, n_groups):
    """Build per-core input maps. Core c = (batch b, head-group g): c = b*n_groups + g."""
    b_sz = x.shape[0]
    dim, hg, sh = cfg.dim, cfg.hg, cfg.shard
    xTs = []
    for b in range(b_sz):
        xt = np.ascontiguousarray(
            x[b].T.reshape(cfg.kc, 128, cfg.n).transpose(1, 0, 2)
        )
        xTs.append(xt)

    def wlayout(w):  # [dim, C] -> [128, kc, C]
        return np.ascontiguousarray(
            w.reshape(cfg.kc, 128, w.shape[1]).transpose(1, 0, 2)
        )

    in_maps = []
    for b in range(b_sz):
        for g in range(n_groups):
            wq = W_qkv[:, sh * g : sh * (g + 1)]
            wk = W_qkv[:, dim + sh * g : dim + sh * (g + 1)]
            wv_cols = W_qkv[:, 2 * dim + sh * g : 2 * dim + sh * (g + 1)]
            wv = np.zeros((dim, cfg.vw), np.float32)
            for h in range(hg):
                wv[:, h * (DH + 1) : h * (DH + 1) + DH] = wv_cols[:, h * DH : (h + 1) * DH]
            wo = W_out[sh * g : sh * (g + 1), :]
            wo_l = np.ascontiguousarray(
                wo.reshape(sh // 128, 128, dim).transpose(1, 0, 2)
            )
            in_maps.append(
                {
                    "xT": xTs[b].astype(cfg.np_dt),
                    "wq": wlayout(wq).astype(cfg.np_dt),
                    "wk": wlayout(wk).astype(cfg.np_dt),
                    "wv": wlayout(wv).astype(cfg.np_dt),
                    "wo": wo_l.astype(cfg.np_dt),
                }
            )
    return in_maps


_NC_CACHE = {}


def kernel(x, W_qkv, W_out, b_out):
    x = np.asarray(x, np.float32)
    W_qkv = np.asarray(W_qkv, np.float32)
    W_out = np.asarray(W_out, np.float32)
    b_out = np.asarray(b_out, np.float32)
    cfg = FULL
    bsz = x.shape[0]
    n_groups = N_CORES // bsz

    if "nc" not in _NC_CACHE:
        _NC_CACHE["nc"] = build_program(cfg)
    nc = _NC_CACHE["nc"]

    in_maps = shard_inputs(cfg, x, W_qkv, W_out, n_groups)
    res = run_bass_kernel_spmd(nc, in_maps, list(range(N_CORES)))

    out = np.zeros((bsz, cfg.n, cfg.dim), np.float32)
    for b in range(bsz):
        for g in range(n_groups):
            out[b] += res.results[b * n_groups + g]["out"].astype(np.float32)
        out[b] += b_out
    return out


# revision 10
# speedup vs baseline: 1.0933x; 1.0933x over previous
"""Multi-head attention (b=2, n=2048, dim=1024, h=16, fp32) on 8 TRN2 NeuronCores.

Sharding: 2 batches x 4 head-groups (4 heads each). Each core computes, for its
batch element and 4 heads: QKV projection, softmax attention, and a partial
output projection (W_out rows of its heads). Host sums the 4 partials per batch
and adds the bias.

Device layout choices (per core):
  - x arrives pre-transposed (host) as xT [128, 8, 2048]  (p=dim%128, kc=dim//128, n)
  - Q^T/K^T computed as [128, 2, 2048]: pair g holds heads 2g (partitions 0-63)
    and 2g+1 (partitions 64-127); row r of pair g = W_qkv column (g*128+r).
  - S^T = K @ Q^T per head via row-tiled (K=64) matmul pairs (the pair runs
    concurrently on the two array row-halves: ~216ns/pair); softmax exp on
    ScalarE directly PSUM->SBUF with scale=dim^-0.5 folded in (no max
    subtraction needed: |scores*scale| < ~0.5).
  - V is augmented with a ones column per head ([V_h | 1]) so the PV matmul's
    65th output row accumulates the softmax denominator for free.
  - Normalization: reciprocal_approx_fast (DVE) + partition_broadcast (GPSIMD)
    + one tensor_tensor multiply fused with the PSUM->SBUF evacuation.
  - Emission schedule: the QKV/out projections are diced into granules of <=4
    matmuls and drizzled into the attention sweeps (one granule per chunk) so
    the PE never starves ScalarE's exp stream and projection LDWEIGHTS mostly
    hide behind adjacent matmuls. Attention starts as soon as the first
    512-token block of x lands; out-projection for block b fills the sweeps
    after its pair-1 sweep completes, leaving only the last block as tail.
  - Optionally every EXP_DVE_EVERY-th chunk computes exp on the Vector engine
    via ((x+2)^2/8 + 0.5)^2 (max rel err ~0.6% on |x|<=0.5) to unload ScalarE.
"""

import os
import numpy as np
from collections import deque
from contextlib import ExitStack
from functools import partial

import concourse.bass as bass
import concourse.mybir as mybir
import concourse.tile as tile
from concourse import bacc
from concourse.bass import ts
from concourse.bass_utils import run_bass_kernel_spmd

F32 = mybir.dt.float32
F32R = mybir.dt.float32r

N_CORES = 8
HEADS = 16
DH = 64  # head dim


class Cfg:
    def __init__(self, n, dim, hg):
        self.n = n                    # sequence length (per core)
        self.dim = dim                # model dim
        self.hg = hg                  # heads per core
        self.kc = dim // 128          # dim chunks of 128
        self.nqb = max(1, n // 512)   # query blocks of 512
        self.qb = min(n, 512)
        self.nkc = n // 128           # key chunks of 128
        self.pairs = hg // 2
        self.shard = hg * DH          # qkv shard columns per section
        self.vw = hg * (DH + 1)       # V columns incl per-head ones col
        self.mm_dt = mybir.dt.float16
        self.np_dt = np.float16


FULL = Cfg(2048, 1024, 4)
# in pair-1 sweeps, every n-th chunk's exp runs on DVE instead of ScalarE (0 = off)
EXP_DVE_EVERY = int(os.environ.get("ATTN_EXP_DVE_EVERY", "0"))


def build_kernel(tc, ctx, cfg, xT, wq, wk, wv, wo, out):
    nc = tc.nc
    P = 128
    KC, NQB, QB, NKC, PAIRS = cfg.kc, cfg.nqb, cfg.qb, cfg.nkc, cfg.pairs
    MD = cfg.mm_dt
    SCALE = cfg.dim ** -0.5
    M_SLABS = cfg.shard // 128  # = PAIRS

    wpool = ctx.enter_context(tc.tile_pool(name="w", bufs=1))
    wq_sb = wpool.tile([P, KC, cfg.shard], MD, tag="wq", name="wq_sb")
    wk_sb = wpool.tile([P, KC, cfg.shard], MD, tag="wk", name="wk_sb")
    wv_sb = wpool.tile([P, KC, cfg.vw], MD, tag="wv", name="wv_sb")
    wo_sb = wpool.tile([P, M_SLABS, cfg.dim], MD, tag="wo", name="wo_sb")

    per = ctx.enter_context(tc.tile_pool(name="per", bufs=1))
    qt = {}  # (pair, nqb) -> [128, QB]
    kt = {}
    vt = {}  # nt -> [128, vw]
    on = {}  # (slab, nqb) -> [128, QB]  normalized O^T for out-proj lhsT
    for g in range(PAIRS):
        for b in range(NQB):
            qt[g, b] = per.tile([P, QB], MD, tag=f"qt{g}_{b}", name=f"qt{g}_{b}")
            kt[g, b] = per.tile([P, QB], MD, tag=f"kt{g}_{b}", name=f"kt{g}_{b}")
            on[g, b] = per.tile([P, QB], MD, tag=f"on{g}_{b}", name=f"on{g}_{b}")
    for t in range(NKC):
        vt[t] = per.tile([P, cfg.vw], MD, tag=f"v{t}", name=f"v{t}")

    xpool = ctx.enter_context(tc.tile_pool(name="x", bufs=1))
    # single-buffer projection psum pools: an emit's accumulation spans several
    # interleaved granules, so each class gets a dedicated bank (no rotation
    # collisions). psS 4 + psO 2 + paA 1 + paV 1 = 8 banks.
    paA = ctx.enter_context(tc.tile_pool(name="paA", bufs=1, space="PSUM"))
    paV = ctx.enter_context(tc.tile_pool(name="paV", bufs=1, space="PSUM"))
    psS = ctx.enter_context(tc.tile_pool(name="psS", bufs=2, space="PSUM"))
    psO = ctx.enter_context(tc.tile_pool(name="psO", bufs=1, space="PSUM"))
    epool = ctx.enter_context(tc.tile_pool(name="e", bufs=8))
    ppool = ctx.enter_context(tc.tile_pool(name="p", bufs=2))
    npool = ctx.enter_context(tc.tile_pool(name="nrm", bufs=3))
    copool = ctx.enter_context(tc.tile_pool(name="co", bufs=6))

    xts = {}
    for b in range(NQB):
        xts[b] = xpool.tile([P, KC, QB], MD, tag=f"xt{b}", name=f"xt{b}")

    # ---- input DMA: first x block + wk lead; everything else follows in
    # halves so the 16 queues run wide and block 0 lands first.
    h = KC // 2
    nc.sync.dma_start(wk_sb[:, :h], wk[:, :h])
    for q4 in range(4):
        nc.sync.dma_start(xts[0][:, ts(q4, 2)], xT[:, ts(q4, 2), ts(0, QB)])
    nc.sync.dma_start(wk_sb[:, h:], wk[:, h:])
    nc.sync.dma_start(wq_sb[:, :h], wq[:, :h])
    nc.sync.dma_start(wq_sb[:, h:], wq[:, h:])
    nc.sync.dma_start(wv_sb[:, :h], wv[:, :h])
    nc.sync.dma_start(wv_sb[:, h:], wv[:, h:])
    for b in range(1, NQB):
        nc.sync.dma_start(xts[b][:, :h], xT[:, :h, ts(b, QB)])
        nc.sync.dma_start(xts[b][:, h:], xT[:, h:, ts(b, QB)])
    nc.sync.dma_start(wo_sb[:], wo[:])

    # ---- granule machinery: pend holds (key, fn) units of <=4 PE matmuls.
    pend = deque()
    emitted = set()

    def fill(budget):
        n = 0
        while pend and n < budget:
            key, fn = pend.popleft()
            fn()
            emitted.add(key)
            n += 1

    def require(key):
        while pend and key not in emitted:
            k, fn = pend.popleft()
            fn()
            emitted.add(k)

    # ---- projection emitters (as granules) ----
    qk_ps = {}

    def qk_part(w_sb, dst, g, b, kcs, evac):
        if kcs[0] == 0:
            qk_ps[0] = paA.tile([P, 512], F32, tag="pa", name="pa")
        ps = qk_ps[0]
        for kc in kcs:
            nc.tensor.matmul(
                ps[:, :QB],
                lhsT=w_sb[:, kc, ts(g, 128)],
                rhs=xts[b][:, kc, :],
                start=(kc == 0),
                stop=(kc == KC - 1),
            )
        if evac:
            nc.vector.tensor_copy(dst[g, b][:], ps[:, :QB])

    def push_qk(which, w_sb, dst, g, b):
        groups = [list(range(0, 3)), list(range(3, 6)), list(range(6, KC))]
        for i, kcs in enumerate(groups):
            key = (which, g, b) if i == len(groups) - 1 else (which, g, b, i)
            pend.append((key, partial(qk_part, w_sb, dst, g, b, kcs, i == len(groups) - 1)))

    v_ps = {}

    def v_part(nt, kcs, evac):
        vb, t = divmod(nt, QB // 128)
        if kcs[0] == 0:
            v_ps[0] = paV.tile([P, 512], F32, tag="pv", name="pv")
        ps = v_ps[0]
        for kc in kcs:
            nc.tensor.matmul(
                ps[:, : cfg.vw],
                lhsT=xts[vb][:, kc, ts(t, 128)],
                rhs=wv_sb[:, kc, :],
                start=(kc == 0),
                stop=(kc == KC - 1),
            )
        if evac:
            nc.vector.tensor_copy(vt[nt][:], ps[:, : cfg.vw])
            v4 = vt[nt][:].rearrange("p (h e) -> p h e", e=DH + 1)
            nc.vector.memset(v4[:, :, DH : DH + 1], 1.0)

    def emit_v(nt):
        v_part(nt, list(range(0, 4)), False)
        v_part(nt, list(range(4, KC)), True)

    # ---- out projection granules: per (bb, t, nh): 2 matmuls + evac + DMA
    def out_part(bb, t, nh, tail):
        nt = bb * (QB // 128) + t
        # alternate between the two projection psum banks (V emits are long
        # done by now) so consecutive out granules pipeline instead of
        # serializing on a single bank's evacuation
        if (t * (cfg.dim // 512) + nh) % 2 == 0:
            ps = paA.tile([P, 512], F32, tag="pa", name="pc")
        else:
            ps = paV.tile([P, 512], F32, tag="pv", name="pcv")
        for kc in range(M_SLABS):
            nc.tensor.matmul(
                ps[:],
                lhsT=on[kc, bb][:, ts(t, 128)],
                rhs=wo_sb[:, kc, ts(nh, 512)],
                start=(kc == 0),
                stop=(kc == M_SLABS - 1),
            )
        ot = copool.tile([P, 512], MD, tag="ot", name="ot")
        if tail:
            # tail blocks: ScalarE is idle after the last exp
            nc.scalar.copy(ot[:], ps[:])
        else:
            nc.vector.tensor_copy(ot[:], ps[:])
        nc.sync.dma_start(out[ts(nt, 128), ts(nh, 512)], ot[:])

    def push_out(bb, tail=False):
        for t in range(QB // 128):
            for nh in range(cfg.dim // 512):
                pend.append(((("o", bb, t, nh)), partial(out_part, bb, t, nh, tail)))

    # ---- attention sweep ----
    sweep_idx = [0]

    def attention(b, g, with_v=False, fe=2, dve_every=0, last=False):
        o_ps = psO.tile([P, 2, 512], F32, tag="o", name="o_ps")
        require(("q", g, b))
        ets = {}

        def pv(c):
            v4 = vt[c][:].rearrange("p (h e) -> p h e", e=DH + 1)
            e_t = ets.pop(c)
            for a in range(2):
                hh = 2 * g + a
                nc.tensor.matmul(
                    o_ps[0 : DH + 1, a, :QB],
                    lhsT=v4[:, hh, :],
                    rhs=e_t[:, a, :QB],
                    start=(c == 0),
                    stop=(c == NKC - 1),
                )

        for c in range(NKC):
            require(("k", g, c * 128 // QB))
            s_ps = psS.tile([P, 2, 512], F32, tag="s", name="s_ps")
            for a in range(2):
                lo = a * 64
                nc.tensor.matmul(
                    s_ps[:, a, :QB],
                    lhsT=kt[g, c * 128 // QB][lo : lo + 64, ts(c % (QB // 128), 128)],
                    rhs=qt[g, b][lo : lo + 64, :],
                    start=True,
                    stop=True,
                )
            # PV for the previous chunk goes to the PE queue here, ahead of
            # this chunk's exp, hiding the exp->PV semaphore handoff
            if c > 0:
                pv(c - 1)
            e_t = epool.tile([P, 2, 512], MD, tag="e", name="e_t")
            ets[c] = e_t
            if dve_every and c % dve_every == dve_every - 1:
                # e = ((x+2)^2 / 8 + 0.5)^2, x = scale * s
                a_t = ppool.tile([P, 2, 512], MD, tag="pe_a", name="pe_a")
                b_t = ppool.tile([P, 2, 512], MD, tag="pe_b", name="pe_b")
                nc.vector.tensor_scalar(
                    a_t[:, :, :QB], s_ps[:, :, :QB], SCALE, 2.0,
                    mybir.AluOpType.mult, mybir.AluOpType.add,
                )
                nc.vector.tensor_tensor(
                    b_t[:, :, :QB], a_t[:, :, :QB], a_t[:, :, :QB], mybir.AluOpType.mult
                )
                nc.vector.tensor_scalar(
                    a_t[:, :, :QB], b_t[:, :, :QB], 0.125, 0.5,
                    mybir.AluOpType.mult, mybir.AluOpType.add,
                )
                nc.vector.tensor_tensor(
                    e_t[:, :, :QB], a_t[:, :, :QB], a_t[:, :, :QB], mybir.AluOpType.mult
                )
            else:
                nc.scalar.activation(
                    e_t[:, :, :QB],
                    s_ps[:, :, :QB],
                    mybir.ActivationFunctionType.Exp,
                    scale=SCALE,
                )
            if with_v and c < NKC - 1:
                emit_v(c + 1)
            if c % (2 * fe) == 2 * fe - 1:
                fill(2)
        pv(NKC - 1)
        sweep_idx[0] += 1
        # one-shot evacuation frees the O psum banks immediately; skipped on
        # the final sweep where nothing waits on the banks
        if last:
            oev = o_ps
        else:
            oev = npool.tile([P, 2, 512], F32, tag="oev", name="oev")
            nc.vector.tensor_copy(oev[0 : DH + 1, :, :QB], o_ps[0 : DH + 1, :, :QB])
        # normalize; stage the denom row at partition 0 (the custom DVE
        # reciprocal misreads inputs at a nonzero base partition). On the last
        # sweep run it in two q-halves so the tail out-proj (which consumes
        # 128-token slabs of `on`) can start after the first half.
        drow = npool.tile([1, 2, 512], F32, tag="drow", name="drow")
        recip = npool.tile([1, 2, 512], F32, tag="recip", name="recip")
        bcast = npool.tile([64, 2, 512], F32, tag="bcast", name="bcast")
        halves = [(0, QB // 2), (QB // 2, QB)] if last else [(0, QB)]
        for qlo, qhi in halves:
            nc.vector.tensor_copy(drow[:, :, qlo:qhi], o_ps[DH : DH + 1, :, qlo:qhi])
            nc.vector.reciprocal_approx_fast(
                out=recip[:, :, qlo:qhi], in_=drow[:, :, qlo:qhi]
            )
            nc.gpsimd.partition_broadcast(bcast[:, :, qlo:qhi], recip[:, :, qlo:qhi])
            for a in range(2):
                nc.vector.tensor_tensor(
                    on[g, b][a * 64 : a * 64 + 64, qlo:qhi],
                    oev[0:DH, a, qlo:qhi],
                    bcast[:, a, qlo:qhi],
                    mybir.AluOpType.mult,
                )

    # ---- emission schedule ----
    # seed: K(0,0) + Q(0,0) emit immediately (block 0 DMA leads); the rest of
    # the projections go through the granule queue, popped one per chunk and
    # force-drained by require() at dependency edges.
    push_qk("k", wk_sb, kt, 0, 0)
    push_qk("q", wq_sb, qt, 0, 0)
    for b in range(1, NQB):
        push_qk("k", wk_sb, kt, 0, b)
    for b in range(1, NQB):
        push_qk("q", wq_sb, qt, 0, b)
    for b in range(NQB):
        push_qk("k", wk_sb, kt, 1, b)
    for b in range(NQB):
        push_qk("q", wq_sb, qt, 1, b)

    emit_v(0)
    attention(0, 0, with_v=True, fe=4)
    for b in range(1, NQB):
        attention(b, 0, fe=3)
    for b in range(NQB):
        attention(b, 1, fe=2, dve_every=EXP_DVE_EVERY, last=(b == NQB - 1))
        # out-proj for block b becomes available now; queue it as filler work
        push_out(b, tail=(b == NQB - 1))
    while pend:
        fill(len(pend))


def build_program(cfg, num_devices=N_CORES):
    nc = bacc.Bacc("TRN2", target_bir_lowering=False, debug=False, num_devices=num_devices)
    P = 128
    xT = nc.dram_tensor("xT", [P, cfg.kc, cfg.n], cfg.mm_dt, kind="ExternalInput").ap()
    wq = nc.dram_tensor("wq", [P, cfg.kc, cfg.shard], cfg.mm_dt, kind="ExternalInput").ap()
    wk = nc.dram_tensor("wk", [P, cfg.kc, cfg.shard], cfg.mm_dt, kind="ExternalInput").ap()
    wv = nc.dram_tensor("wv", [P, cfg.kc, cfg.vw], cfg.mm_dt, kind="ExternalInput").ap()
    wo = nc.dram_tensor("wo", [P, cfg.shard // 128, cfg.dim], cfg.mm_dt, kind="ExternalInput").ap()
    out = nc.dram_tensor("out", [cfg.n, cfg.dim], cfg.mm_dt, kind="ExternalOutput").ap()
    with tile.TileContext(nc) as tc, ExitStack() as ctx:
        build_kernel(tc, ctx, cfg, xT, wq, wk, wv, wo, out)
    nc.compile()
    return nc


def shard_inputs(cfg, x, W_qkv, W_out, n_groups):
    """Build per-core input maps. Core c = (batch b, head-group g): c = b*n_groups + g."""
    b_sz = x.shape[0]
    dim, hg, sh = cfg.dim, cfg.hg, cfg.shard
    xTs = []
    for b in range(b_sz):
        xt = np.ascontiguousarray(
            x[b].T.reshape(cfg.kc, 128, cfg.n).transpose(1, 0, 2)
        )
        xTs.append(xt)

    def wlayout(w):  # [dim, C] -> [128, kc, C]
        return np.ascontiguousarray(
            w.reshape(cfg.kc, 128, w.shape[1]).transpose(1, 0, 2)
        )

    in_maps = []
    for b in range(b_sz):
        for g in range(n_groups):
            wq = W_qkv[:, sh * g : sh * (g + 1)]
            wk = W_qkv[:, dim + sh * g : dim + sh * (g + 1)]
            wv_cols = W_qkv[:, 2 * dim + sh * g : 2 * dim + sh * (g + 1)]
            wv = np.zeros((dim, cfg.vw), np.float32)
            for h in range(hg):
                wv[:, h * (DH + 1) : h * (DH + 1) + DH] = wv_cols[:, h * DH : (h + 1) * DH]
            wo = W_out[sh * g : sh * (g + 1), :]
            wo_l = np.ascontiguousarray(
                wo.reshape(sh // 128, 128, dim).transpose(1, 0, 2)
            )
            in_maps.append(
                {
                    "xT": xTs[b].astype(cfg.np_dt),
                    "wq": wlayout(wq).astype(cfg.np_dt),
                    "wk": wlayout(wk).astype(cfg.np_dt),
                    "wv": wlayout(wv).astype(cfg.np_dt),
                    "wo": wo_l.astype(cfg.np_dt),
                }
            )
    return in_maps


_NC_CACHE = {}


def kernel(x, W_qkv, W_out, b_out):
    x = np.asarray(x, np.float32)
    W_qkv = np.asarray(W_qkv, np.float32)
    W_out = np.asarray(W_out, np.float32)
    b_out = np.asarray(b_out, np.float32)
    cfg = FULL
    bsz = x.shape[0]
    n_groups = N_CORES // bsz

    if "nc" not in _NC_CACHE:
        _NC_CACHE["nc"] = build_program(cfg)
    nc = _NC_CACHE["nc"]

    in_maps = shard_inputs(cfg, x, W_qkv, W_out, n_groups)
    res = run_bass_kernel_spmd(nc, in_maps, list(range(N_CORES)))

    out = np.zeros((bsz, cfg.n, cfg.dim), np.float32)
    for b in range(bsz):
        for g in range(n_groups):
            out[b] += res.results[b * n_groups + g]["out"].astype(np.float32)
        out[b] += b_out
    return out
